# revision 36
# baseline (speedup 1.0000x reference)
"""DeepseekV3 TopK router kernel for 8 Trainium2 NeuronCores.

Reference computation (per token):
    router_logits = hidden @ weight.T          # (B*S, 64) fp32
    topk_vals, topk_idx = top_k(logits, 8)     # descending
    topk_w = softmax(topk_vals)

Sharding: data-parallel over batch*seq (16384 tokens -> 2048/core).
Each core's hidden shard is staged H-major (transposed on host) so the
contraction dim H lands on SBUF partitions; the small router weight is
replicated (transposed to (H, E) on host).

Device kernel per core (raw bass, explicit semaphores):
  - tokens processed in 2 half-passes of 1024; per half, 32 DMA loads of
    [128h, 1024t] fp32 hidden tiles (512 KiB each, ring of 8)
  - matmul orientation: hidden tile slice [128h, 128t] is the STATIONARY
    operand, weightT chunk [128h, 64e] streams -> fp32 cost scales with
    the 64-wide moving operand; logits accumulate token-major
    [128t, 64e] across the 32 h-chunks in 8 PSUM banks (one per token
    subtile of 128)
  - tail per subtile: DVE packs the expert id into the low 6 mantissa
    bits of each logit (straight from PSUM), MAX8 gives the top-8
    descending, bitwise extracts recover indices + values, ACT Exp
    (+accumulated row sum) and DVE reciprocal/scale form the softmax.
  - DVE pipelines overlap back-to-back instructions: nc.vector.drain()
    guards every same-engine RAW boundary.
"""

import numpy as np

import concourse.bass as bass
from concourse import mybir
from concourse.bass_utils import run_bass_kernel_spmd

N_CORES = 8
B, S, H, E, TOPK = 4, 4096, 4096, 64, 8
T = B * S            # 16384 tokens total
TC = T // N_CORES    # 2048 tokens per core
NH = H // 128        # 32 h-chunks
HALVES = 2
THALF = TC // HALVES  # 1024 tokens per half-pass
NM = THALF // 128     # 8 token subtiles (PSUM banks) per half
NG = TC // 128        # 16 token subtiles per core
NCC = NH // 2         # 16 paired-chunk DMAs per half (1 MiB each)
NB2 = 6               # paired-chunk ring depth (each slot = 2 h-chunks)

F32 = mybir.dt.float32


def _build_nc(reps=1):
    nc = bass.Bass("TRN2", target_bir_lowering=False, debug=False)

    hid_t = nc.dram_tensor("hid_t", [H, TC], F32, kind="ExternalInput").ap()
    w_t = nc.dram_tensor("w_t", [H, E], F32, kind="ExternalInput").ap()
    iota = nc.dram_tensor("iota", [128, E], mybir.dt.uint32, kind="ExternalInput").ap()
    lg_out = nc.dram_tensor("logits", [TC, E], F32, kind="ExternalOutput").ap()
    ix_out = nc.dram_tensor("topk_idx", [TC, TOPK], mybir.dt.int32, kind="ExternalOutput").ap()
    tw_out = nc.dram_tensor("topk_w", [TC, TOPK], F32, kind="ExternalOutput").ap()

    hid_sb = nc.alloc_sbuf_tensor("hid_sb", [128, NB2, 2, THALF], F32).ap()
    wt_sb = nc.alloc_sbuf_tensor("wt_sb", [128, NH, E], F32).ap()
    iota_sb = nc.alloc_sbuf_tensor("iota_sb", [128, E], mybir.dt.uint32).ap()
    lg_sb = nc.alloc_sbuf_tensor("lg_sb", [128, NG, E], F32).ap()
    lgp_sb = nc.alloc_sbuf_tensor("lgp_sb", [128, NM, E], mybir.dt.uint32).ap()
    valsp_sb = nc.alloc_sbuf_tensor("valsp_sb", [128, NG, TOPK], F32).ap()
    evals_sb = nc.alloc_sbuf_tensor("evals_sb", [128, NG, TOPK], F32).ap()
    idx_sb = nc.alloc_sbuf_tensor("idx_sb", [128, NG, TOPK], mybir.dt.uint32).ap()
    w_sb = nc.alloc_sbuf_tensor("w_sb", [128, NG, TOPK], F32).ap()
    neg_sb = nc.alloc_sbuf_tensor("neg_sb", [128, NG], F32).ap()
    sum_sb = nc.alloc_sbuf_tensor("sum_sb", [128, NG], F32).ap()
    rsum_sb = nc.alloc_sbuf_tensor("rsum_sb", [128, NG], F32).ap()
    mask_hi_sb = nc.alloc_sbuf_tensor("mask_hi_sb", [128, 1], mybir.dt.uint32).ap()
    mask_lo_sb = nc.alloc_sbuf_tensor("mask_lo_sb", [128, 1], mybir.dt.uint32).ap()
    zero8_sb = nc.alloc_sbuf_tensor("zero8_sb", [128, TOPK], mybir.dt.uint32).ap()

    ps_m = [nc.alloc_psum_tensor(f"ps_m{j}", [128, E], F32).ap() for j in range(NM)]

    hs_sems = []

    with (
        nc.Block() as block,
        nc.semaphore("wt_sem") as wt_sem,        # weight DMA (first half)
        nc.semaphore("wt2_sem") as wt2_sem,      # weight DMA (second half)
        nc.semaphore("io_sem") as io_sem,        # iota DMA
        nc.semaphore("pe_sem") as pe_sem,        # per-h-chunk matmul completion
        nc.semaphore("pk_sem") as pk_sem,        # DVE done reading a PSUM bank
        nc.semaphore("ng_sem") as ng_sem,        # DVE neg-max ready (per subtile)
        nc.semaphore("ev_sem") as ev_sem,        # ACT exp done (per subtile)
        nc.semaphore("dv_sem") as dv_sem,        # DVE subtile fully done
        nc.semaphore("o1_sem") as o1_sem,        # output DMAs (one sem each:
        nc.semaphore("o2_sem") as o2_sem,        #  multi-DMA increments on one
        nc.semaphore("o3_sem") as o3_sem,        #  sem complete out of order)
    ):
        for j in range(NB2):
            hs_sems.append(nc.alloc_semaphore(f"hs{j}_sem"))

        @block.sync
        def _(sync):
            for it in range(reps):
                for half in range(HALVES):
                    hg = it * HALVES + half
                    t0 = half * THALF
                    for cc in range(NCC):
                        pg = hg * NCC + cc
                        if pg >= NB2:
                            sync.wait_ge(pe_sem, 2 * (pg - NB2 + 1))
                        sync.dma_start(
                            hid_sb[:, pg % NB2, :, :],
                            hid_t[cc * 256:(cc + 1) * 256, t0:t0 + THALF].rearrange("(two p) t -> p two t", p=128),
                        ).then_inc(hs_sems[pg % NB2], 16)
            sync.wait_ge(o1_sem, 16 * HALVES * reps)
            sync.wait_ge(o2_sem, 16 * HALVES * reps)
            sync.wait_ge(o3_sem, 16 * HALVES * reps)

        @block.tensor
        def _(tensor):
            tensor.wait_ge(wt_sem, 16)
            wt_full_waited = [False]
            for it in range(reps):
                for half in range(HALVES):
                    hg = it * HALVES + half
                    if hg >= 1:
                        # previous half's PSUM banks consumed by DVE before
                        # start=True clears them
                        tensor.wait_ge(pk_sem, NM * hg)
                    for cc in range(NCC):
                        pg = hg * NCC + cc
                        tensor.wait_ge(hs_sems[pg % NB2], 16 * (pg // NB2 + 1))
                        if cc * 2 >= NH // 2 and not wt_full_waited[0]:
                            tensor.wait_ge(wt2_sem, 16)
                            wt_full_waited[0] = True
                        for two in range(2):
                            c = 2 * cc + two
                            for j in range(NM):
                                mm = nc.tensor.matmul(
                                    ps_m[j],
                                    hid_sb[:, pg % NB2, two, j * 128:(j + 1) * 128],
                                    wt_sb[:, c, :],
                                    start=(c == 0),
                                    stop=(c == NH - 1),
                                )
                                if j == NM - 1:
                                    mm.then_inc(pe_sem, 1)

        @block.vector
        def _(vector):
            U32 = mybir.dt.uint32
            PACK_MASK = 0xFFFFFFC0  # clear low 6 mantissa bits -> room for the expert id
            nc.vector.memset(mask_hi_sb, PACK_MASK)
            nc.vector.memset(mask_lo_sb, 63)
            nc.vector.memset(zero8_sb, 0)
            nc.vector.drain()
            vector.wait_ge(io_sem, 16)
            for it in range(reps):
                if it >= 1:
                    # previous rep's output DMAs done before overwriting staging
                    vector.wait_ge(o1_sem, 16 * HALVES * it)
                    vector.wait_ge(o2_sem, 16 * HALVES * it)
                    vector.wait_ge(o3_sem, 16 * HALVES * it)
                for half in range(HALVES):
                    hg = it * HALVES + half
                    vector.wait_ge(pe_sem, NH * (hg + 1))
                    for j in range(NM):
                        g = half * NM + j
                        nc.vector.tensor_copy(lg_sb[:, g, :], ps_m[j])
                        # pack expert index into low mantissa bits: (logit & mask) | e
                        nc.vector.scalar_tensor_tensor(
                            lgp_sb[:, j, :],
                            ps_m[j].bitcast(U32),
                            mask_hi_sb,
                            iota_sb,
                            op0=mybir.AluOpType.bitwise_and,
                            op1=mybir.AluOpType.bitwise_or,
                        ).then_inc(pk_sem, 1)
                    nc.vector.drain()
                    for j in range(NM):
                        g = half * NM + j
                        nc.vector.max(valsp_sb[:, g, :], lgp_sb[:, j, :].bitcast(F32))
                    nc.vector.drain()
                    for j in range(NM):
                        g = half * NM + j
                        nc.vector.scalar_tensor_tensor(
                            idx_sb[:, g, :],
                            valsp_sb[:, g, :].bitcast(U32),
                            mask_lo_sb,
                            zero8_sb,
                            op0=mybir.AluOpType.bitwise_and,
                            op1=mybir.AluOpType.bitwise_or,
                        )
                        # softmax runs on the packed values (2^-18 off the true
                        # logits — well inside tolerance)
                        nc.vector.tensor_scalar_mul(neg_sb[:, g:g + 1], valsp_sb[:, g, 0:1], -1.0).then_inc(ng_sem, 1)
                    for j in range(NM):
                        g = half * NM + j
                        gg = it * NG + g
                        vector.wait_ge(ev_sem, gg + 1)
                        nc.vector.reciprocal(rsum_sb[:, g:g + 1], sum_sb[:, g:g + 1])
                    nc.vector.drain()
                    for j in range(NM):
                        g = half * NM + j
                        nc.vector.tensor_scalar_mul(w_sb[:, g, :], evals_sb[:, g, :], rsum_sb[:, g:g + 1]).then_inc(dv_sem, 1)

        @block.scalar
        def _(scalar):
            # issue the small input loads from ACT's HWDGE ring so they
            # overlap the hidden stream on SP's ring
            wt_re = w_t.rearrange("(c p) e -> p c e", p=128)
            nc.scalar.dma_start(wt_sb[:, :NH // 2, :], wt_re[:, :NH // 2, :]).then_inc(wt_sem, 16)
            nc.scalar.dma_start(wt_sb[:, NH // 2:, :], wt_re[:, NH // 2:, :]).then_inc(wt2_sem, 16)
            nc.scalar.dma_start(iota_sb, iota).then_inc(io_sem, 16)
            for it in range(reps):
                for half in range(HALVES):
                    for j in range(NM):
                        g = half * NM + j
                        gg = it * NG + g
                        scalar.wait_ge(ng_sem, gg + 1)
                        nc.scalar.activation(
                            evals_sb[:, g, :],
                            valsp_sb[:, g, :],
                            mybir.ActivationFunctionType.Exp,
                            bias=neg_sb[:, g:g + 1],
                            scale=1.0,
                            accum_out=sum_sb[:, g:g + 1],
                        ).then_inc(ev_sem, 1)
                    # flush this half's outputs from ACT's ring while SP keeps
                    # streaming the next half's hidden tiles
                    t0 = half * THALF
                    gs = half * NM
                    scalar.wait_ge(dv_sem, NG * it + NM * (half + 1))
                    nc.scalar.dma_start(
                        lg_out[t0:t0 + THALF, :].rearrange("(g p) e -> p g e", p=128),
                        lg_sb[:, gs:gs + NM, :],
                    ).then_inc(o1_sem, 16)
                    nc.scalar.dma_start(
                        ix_out[t0:t0 + THALF, :].rearrange("(g p) k -> p g k", p=128),
                        idx_sb[:, gs:gs + NM, :].bitcast(mybir.dt.int32),
                    ).then_inc(o2_sem, 16)
                    nc.scalar.dma_start(
                        tw_out[t0:t0 + THALF, :].rearrange("(g p) k -> p g k", p=128),
                        w_sb[:, gs:gs + NM, :],
                    ).then_inc(o3_sem, 16)

    return nc


_NC = None


def _get_nc():
    global _NC
    if _NC is None:
        _NC = _build_nc()
    return _NC


def kernel(hidden_states, weight, top_k):
    assert int(top_k) == TOPK
    hs = np.ascontiguousarray(np.asarray(hidden_states, dtype=np.float32)).reshape(T, H)
    w = np.asarray(weight, dtype=np.float32)
    w_t = np.ascontiguousarray(w.T)
    iota = np.tile(np.arange(E, dtype=np.uint32), (128, 1))

    in_maps = []
    for c in range(N_CORES):
        shard = np.ascontiguousarray(hs[c * TC:(c + 1) * TC, :].T)
        in_maps.append({"hid_t": shard, "w_t": w_t, "iota": iota})

    res = run_bass_kernel_spmd(_get_nc(), in_maps, core_ids=list(range(N_CORES)))
    r = res.results
    logits = np.concatenate([r[c]["logits"] for c in range(N_CORES)], axis=0).reshape(B, S, E)
    idx = np.concatenate([r[c]["topk_idx"] for c in range(N_CORES)], axis=0).reshape(B, S, TOPK)
    tw = np.concatenate([r[c]["topk_w"] for c in range(N_CORES)], axis=0).reshape(B, S, TOPK)
    return logits, idx, tw


# revision 41
# speedup vs baseline: 1.0181x; 1.0181x over previous
"""DeepseekV3 TopK router kernel for 8 Trainium2 NeuronCores.

Reference computation (per token):
    router_logits = hidden @ weight.T          # (B*S, 64) fp32
    topk_vals, topk_idx = top_k(logits, 8)     # descending
    topk_w = softmax(topk_vals)

Sharding: data-parallel over batch*seq (16384 tokens -> 2048/core).
Each core's hidden shard is staged H-major (transposed on host) so the
contraction dim H lands on SBUF partitions; the small router weight is
replicated (transposed to (H, E) on host).

Device kernel per core (raw bass, explicit semaphores):
  - tokens processed in 2 half-passes of 1024; per half, 32 DMA loads of
    [128h, 1024t] fp32 hidden tiles (512 KiB each, ring of 8)
  - matmul orientation: hidden tile slice [128h, 128t] is the STATIONARY
    operand, weightT chunk [128h, 64e] streams -> fp32 cost scales with
    the 64-wide moving operand; logits accumulate token-major
    [128t, 64e] across the 32 h-chunks in 8 PSUM banks (one per token
    subtile of 128)
  - tail per subtile: DVE packs the expert id into the low 6 mantissa
    bits of each logit (straight from PSUM), MAX8 gives the top-8
    descending, bitwise extracts recover indices + values, ACT Exp
    (+accumulated row sum) and DVE reciprocal/scale form the softmax.
  - DVE pipelines overlap back-to-back instructions: nc.vector.drain()
    guards every same-engine RAW boundary.
"""

import numpy as np

import concourse.bass as bass
from concourse import mybir
from concourse.bass_utils import run_bass_kernel_spmd

N_CORES = 8
B, S, H, E, TOPK = 4, 4096, 4096, 64, 8
T = B * S            # 16384 tokens total
TC = T // N_CORES    # 2048 tokens per core
NH = H // 128        # 32 h-chunks
HALVES = 2
THALF = TC // HALVES  # 1024 tokens per half-pass
NM = THALF // 128     # 8 token subtiles (PSUM banks) per half
NG = TC // 128        # 16 token subtiles per core
NCC = NH // 2         # 16 paired-chunk DMAs per half (1 MiB each)
NB2 = 8               # paired-chunk ring depth (each slot = 2 h-chunks)

F32 = mybir.dt.float32


def _build_nc(reps=1):
    nc = bass.Bass("TRN2", target_bir_lowering=False, debug=False)

    hid_t = nc.dram_tensor("hid_t", [H, TC], F32, kind="ExternalInput").ap()
    w_t = nc.dram_tensor("w_t", [H, E], F32, kind="ExternalInput").ap()
    iota = nc.dram_tensor("iota", [128, E], mybir.dt.uint32, kind="ExternalInput").ap()
    lg_out = nc.dram_tensor("logits", [TC, E], F32, kind="ExternalOutput").ap()
    ix_out = nc.dram_tensor("topk_idx", [TC, TOPK], mybir.dt.int32, kind="ExternalOutput").ap()
    tw_out = nc.dram_tensor("topk_w", [TC, TOPK], F32, kind="ExternalOutput").ap()

    hid_sb = nc.alloc_sbuf_tensor("hid_sb", [128, NB2, 2, THALF], F32).ap()
    wt_sb = nc.alloc_sbuf_tensor("wt_sb", [128, NH, E], F32).ap()
    iota_sb = nc.alloc_sbuf_tensor("iota_sb", [128, E], mybir.dt.uint32).ap()
    lg_sb = nc.alloc_sbuf_tensor("lg_sb", [128, NG, E], F32).ap()
    lgp_sb = nc.alloc_sbuf_tensor("lgp_sb", [128, NM, E], mybir.dt.uint32).ap()
    valsp_sb = nc.alloc_sbuf_tensor("valsp_sb", [128, NG, TOPK], F32).ap()
    evals_sb = nc.alloc_sbuf_tensor("evals_sb", [128, NG, TOPK], F32).ap()
    idx_sb = nc.alloc_sbuf_tensor("idx_sb", [128, NG, TOPK], mybir.dt.uint32).ap()
    w_sb = nc.alloc_sbuf_tensor("w_sb", [128, NG, TOPK], F32).ap()
    sub_sb = nc.alloc_sbuf_tensor("sub_sb", [128, NG, TOPK], F32).ap()
    sum_sb = nc.alloc_sbuf_tensor("sum_sb", [128, NG], F32).ap()
    rsum_sb = nc.alloc_sbuf_tensor("rsum_sb", [128, NG], F32).ap()
    mask_hi_sb = nc.alloc_sbuf_tensor("mask_hi_sb", [128, 1], mybir.dt.uint32).ap()
    mask_lo_sb = nc.alloc_sbuf_tensor("mask_lo_sb", [128, 1], mybir.dt.uint32).ap()
    zero8_sb = nc.alloc_sbuf_tensor("zero8_sb", [128, TOPK], mybir.dt.uint32).ap()

    ps_m = [nc.alloc_psum_tensor(f"ps_m{j}", [128, E], F32).ap() for j in range(NM)]

    hs_sems = []

    with (
        nc.Block() as block,
        nc.semaphore("wt_sem") as wt_sem,        # weight DMA (first half)
        nc.semaphore("wt2_sem") as wt2_sem,      # weight DMA (second half)
        nc.semaphore("io_sem") as io_sem,        # iota DMA
        nc.semaphore("pe_sem") as pe_sem,        # per-h-chunk matmul completion
        nc.semaphore("pk_sem") as pk_sem,        # DVE done reading a PSUM bank
        nc.semaphore("ng_sem") as ng_sem,        # DVE neg-max ready (per subtile)
        nc.semaphore("ev_sem") as ev_sem,        # ACT exp done (per subtile)
        nc.semaphore("dv_sem") as dv_sem,        # DVE subtile fully done
        nc.semaphore("o1_sem") as o1_sem,        # output DMAs (one sem each:
        nc.semaphore("o2_sem") as o2_sem,        #  multi-DMA increments on one
        nc.semaphore("o3_sem") as o3_sem,        #  sem complete out of order)
    ):
        for j in range(NB2):
            hs_sems.append(nc.alloc_semaphore(f"hs{j}_sem"))

        @block.sync
        def _(sync):
            for it in range(reps):
                for half in range(HALVES):
                    hg = it * HALVES + half
                    t0 = half * THALF
                    for cc in range(NCC):
                        pg = hg * NCC + cc
                        if pg >= NB2:
                            sync.wait_ge(pe_sem, 2 * (pg - NB2 + 1))
                        sync.dma_start(
                            hid_sb[:, pg % NB2, :, :],
                            hid_t[cc * 256:(cc + 1) * 256, t0:t0 + THALF].rearrange("(two p) t -> p two t", p=128),
                        ).then_inc(hs_sems[pg % NB2], 16)
            sync.wait_ge(o1_sem, 16 * HALVES * reps)
            sync.wait_ge(o2_sem, 16 * HALVES * reps)
            sync.wait_ge(o3_sem, 16 * HALVES * reps)

        @block.tensor
        def _(tensor):
            tensor.wait_ge(wt_sem, 16)
            wt_full_waited = [False]
            for it in range(reps):
                for half in range(HALVES):
                    hg = it * HALVES + half
                    for cc in range(NCC):
                        pg = hg * NCC + cc
                        tensor.wait_ge(hs_sems[pg % NB2], 16 * (pg // NB2 + 1))
                        if cc * 2 >= NH // 2 and not wt_full_waited[0]:
                            tensor.wait_ge(wt2_sem, 16)
                            wt_full_waited[0] = True
                        for two in range(2):
                            c = 2 * cc + two
                            for j in range(NM):
                                if hg >= 1 and c == 0:
                                    # bank j consumed by DVE (packed) before
                                    # start=True clears it
                                    tensor.wait_ge(pk_sem, NM * (hg - 1) + j + 1)
                                mm = nc.tensor.matmul(
                                    ps_m[j],
                                    hid_sb[:, pg % NB2, two, j * 128:(j + 1) * 128],
                                    wt_sb[:, c, :],
                                    start=(c == 0),
                                    stop=(c == NH - 1),
                                )
                                if j == NM - 1:
                                    mm.then_inc(pe_sem, 1)

        @block.vector
        def _(vector):
            U32 = mybir.dt.uint32
            PACK_MASK = 0xFFFFFFC0  # clear low 6 mantissa bits -> room for the expert id
            nc.vector.memset(mask_hi_sb, PACK_MASK)
            nc.vector.memset(mask_lo_sb, 63)
            nc.vector.memset(zero8_sb, 0)
            nc.vector.drain()
            vector.wait_ge(io_sem, 16)
            for it in range(reps):
                if it >= 1:
                    # previous rep's output DMAs done before overwriting staging
                    vector.wait_ge(o1_sem, 16 * HALVES * it)
                    vector.wait_ge(o2_sem, 16 * HALVES * it)
                    vector.wait_ge(o3_sem, 16 * HALVES * it)
                for half in range(HALVES):
                    hg = it * HALVES + half
                    vector.wait_ge(pe_sem, NH * (hg + 1))
                    for j in range(NM):
                        g = half * NM + j
                        nc.vector.tensor_copy(lg_sb[:, g, :], ps_m[j])
                        # pack expert index into low mantissa bits: (logit & mask) | e
                        nc.vector.scalar_tensor_tensor(
                            lgp_sb[:, j, :],
                            ps_m[j].bitcast(U32),
                            mask_hi_sb,
                            iota_sb,
                            op0=mybir.AluOpType.bitwise_and,
                            op1=mybir.AluOpType.bitwise_or,
                        ).then_inc(pk_sem, 1)
                    nc.vector.drain()
                    for j in range(NM):
                        g = half * NM + j
                        nc.vector.max(valsp_sb[:, g, :], lgp_sb[:, j, :].bitcast(F32))
                    nc.vector.drain()
                    gs = half * NM
                    for j in range(NM):
                        g = gs + j
                        nc.vector.scalar_tensor_tensor(
                            idx_sb[:, g, :],
                            valsp_sb[:, g, :].bitcast(U32),
                            mask_lo_sb,
                            zero8_sb,
                            op0=mybir.AluOpType.bitwise_and,
                            op1=mybir.AluOpType.bitwise_or,
                        )
                    # batched softmax on the packed values (2^-18 off the true
                    # logits — well inside tolerance): one sub/exp/reduce/
                    # recip/mul per half instead of per subtile
                    nc.vector.tensor_sub(
                        sub_sb[:, gs:gs + NM, :],
                        valsp_sb[:, gs:gs + NM, :],
                        valsp_sb[:, gs:gs + NM, 0:1].to_broadcast([128, NM, TOPK]),
                    ).then_inc(ng_sem, 1)
                    vector.wait_ge(ev_sem, hg + 1)
                    nc.vector.tensor_reduce(
                        sum_sb[:, gs:gs + NM],
                        evals_sb[:, gs:gs + NM, :],
                        axis=mybir.AxisListType.X,
                        op=mybir.AluOpType.add,
                    )
                    nc.vector.drain()
                    nc.vector.reciprocal(rsum_sb[:, gs:gs + NM], sum_sb[:, gs:gs + NM])
                    nc.vector.drain()
                    nc.vector.tensor_mul(
                        w_sb[:, gs:gs + NM, :],
                        evals_sb[:, gs:gs + NM, :],
                        rsum_sb[:, gs:gs + NM].rearrange("p (m one) -> p m one", one=1).to_broadcast([128, NM, TOPK]),
                    ).then_inc(dv_sem, 1)

        @block.scalar
        def _(scalar):
            # issue the small input loads from ACT's HWDGE ring so they
            # overlap the hidden stream on SP's ring
            wt_re = w_t.rearrange("(c p) e -> p c e", p=128)
            nc.scalar.dma_start(wt_sb[:, :NH // 2, :], wt_re[:, :NH // 2, :]).then_inc(wt_sem, 16)
            nc.scalar.dma_start(wt_sb[:, NH // 2:, :], wt_re[:, NH // 2:, :]).then_inc(wt2_sem, 16)
            nc.scalar.dma_start(iota_sb, iota).then_inc(io_sem, 16)
            for it in range(reps):
                for half in range(HALVES):
                    hg = it * HALVES + half
                    gs = half * NM
                    scalar.wait_ge(ng_sem, hg + 1)
                    nc.scalar.activation(
                        evals_sb[:, gs:gs + NM, :],
                        sub_sb[:, gs:gs + NM, :],
                        mybir.ActivationFunctionType.Exp,
                    ).then_inc(ev_sem, 1)
                    # flush this half's outputs from ACT's ring while SP keeps
                    # streaming the next half's hidden tiles
                    t0 = half * THALF
                    scalar.wait_ge(dv_sem, HALVES * it + half + 1)
                    nc.scalar.dma_start(
                        lg_out[t0:t0 + THALF, :].rearrange("(g p) e -> p g e", p=128),
                        lg_sb[:, gs:gs + NM, :],
                    ).then_inc(o1_sem, 16)
                    nc.scalar.dma_start(
                        ix_out[t0:t0 + THALF, :].rearrange("(g p) k -> p g k", p=128),
                        idx_sb[:, gs:gs + NM, :].bitcast(mybir.dt.int32),
                    ).then_inc(o2_sem, 16)
                    nc.scalar.dma_start(
                        tw_out[t0:t0 + THALF, :].rearrange("(g p) k -> p g k", p=128),
                        w_sb[:, gs:gs + NM, :],
                    ).then_inc(o3_sem, 16)

    return nc


_NC = None


def _get_nc():
    global _NC
    if _NC is None:
        _NC = _build_nc()
    return _NC


def kernel(hidden_states, weight, top_k):
    assert int(top_k) == TOPK
    hs = np.ascontiguousarray(np.asarray(hidden_states, dtype=np.float32)).reshape(T, H)
    w = np.asarray(weight, dtype=np.float32)
    w_t = np.ascontiguousarray(w.T)
    iota = np.tile(np.arange(E, dtype=np.uint32), (128, 1))

    in_maps = []
    for c in range(N_CORES):
        shard = np.ascontiguousarray(hs[c * TC:(c + 1) * TC, :].T)
        in_maps.append({"hid_t": shard, "w_t": w_t, "iota": iota})

    res = run_bass_kernel_spmd(_get_nc(), in_maps, core_ids=list(range(N_CORES)))
    r = res.results
    logits = np.concatenate([r[c]["logits"] for c in range(N_CORES)], axis=0).reshape(B, S, E)
    idx = np.concatenate([r[c]["topk_idx"] for c in range(N_CORES)], axis=0).reshape(B, S, TOPK)
    tw = np.concatenate([r[c]["topk_w"] for c in range(N_CORES)], axis=0).reshape(B, S, TOPK)
    return logits, idx, tw
